# revision 15
# baseline (speedup 1.0000x reference)
"""FFM layer (field-aware factorization machine) on 8 Trainium2 cores.

Strategy: data-parallel over batch (2048 samples/core). The embedding tables
are re-laid-out on the host into one row per global vocab id g (owned by
exactly one field c = g // V): the 19 *other* fields' embeddings for that id,
plus the w_sparse value, padded to 384 bf16 (768 B, the dma_gather 256 B
granularity). Each (sample, field) lookup is then one contiguous gather row.

The gather uses nc.gpsimd.dma_gather (int16 indices). Indices must fit int16,
so gathers address vocab windows of 3 fields (3*10000 < 32767), with
window-relative indices. One gather instruction covers 3 fields x 4
batch-tiles = 1536 rows.

The FFM cross term for pair (i, j>i) is dot(row_i[block j], row_j[block i]);
per 128-sample tile it is computed as 19 fused multiply + per-partition-reduce
vector ops (scalar_tensor_tensor with accum_out). w_sparse sum and the final
reduction + sigmoid run on the scalar engine; the dense linear part is one
more fused vector op against a replicated weight vector.
"""

import os
import sys

import numpy as np


def _import_concourse():
    try:
        import concourse  # noqa: F401
    except ImportError:
        for p in ("/opt/trn_rl_repo", "/root/.axon_site/_ro/trn_rl_repo"):
            if os.path.isdir(p) and p not in sys.path:
                sys.path.insert(0, p)
    import concourse.bass as bass  # noqa: F401
    import concourse.mybir as mybir  # noqa: F401
    import concourse.tile as tile  # noqa: F401
    from concourse import bass_utils  # noqa: F401

    return bass, mybir, tile, bass_utils


# Problem constants (hardcoded per contract)
F = 20          # sparse fields
V = 10000       # vocab per field
VTOT = F * V    # 200000
D = 16          # embed dim
B = 16384       # batch
DD = 13         # dense features
N_CORES = 8
P = 128         # SBUF partitions

BPC = B // N_CORES          # 2048 samples per core
N_TILES = BPC // P          # 16 tiles of 128 samples
ROW = 384                   # gather row (bf16): 19*16 emb + wsp + pad (768 B)
EMB = (F - 1) * D           # 304
WSP = EMB                   # w_sparse slot index
N_G = 4                     # batch-tiles per gather group
COLS_PER_WIN = 3            # fields per gather window (3*V < int16 max)
SINGLE_PACKET = False

WINDOWS = [
    (c0, min(COLS_PER_WIN, F - c0)) for c0 in range(0, F, COLS_PER_WIN)
]
NW = len(WINDOWS)
MAXW = COLS_PER_WIN * 8 * N_G  # idx columns per (group, window) incl. 8x wrap


def _build_program(n_tiles=N_TILES, vtot=VTOT, n_g=N_G, for_sim=False):
    bass, mybir, tile, _ = _import_concourse()

    v = vtot // F
    n_groups = n_tiles // n_g
    assert n_tiles % n_g == 0

    import concourse.bacc as bacc

    # Bacc (not plain Bass): its compile() runs generate_event_semaphores,
    # which splits multi-semaphore waits into InstEventSemaphore prefixes —
    # TRN2 instructions can carry only one inline wait — and inserts the
    # GPSIMD ucode library loads that dma_gather needs.
    nc = bacc.Bacc(None, target_bir_lowering=False, debug=for_sim)

    f32 = mybir.dt.float32
    bf16 = mybir.dt.bfloat16
    i16 = mybir.dt.int16
    mult = mybir.AluOpType.mult
    copy_f = mybir.ActivationFunctionType.Copy
    sigm_f = mybir.ActivationFunctionType.Sigmoid

    t2 = nc.dram_tensor("t2", [vtot, ROW], bf16, kind="ExternalInput")
    idxs = nc.dram_tensor("idxs", [P, n_groups, NW, MAXW], i16, kind="ExternalInput")
    dense_d = nc.dram_tensor("dense", [P, n_tiles, DD + 1], f32, kind="ExternalInput")
    wvec_d = nc.dram_tensor("wvec", [P, DD + 1], f32, kind="ExternalInput")
    out = nc.dram_tensor("out", [P, n_tiles], f32, kind="ExternalOutput")

    with tile.TileContext(nc) as tc:
        with (
            tc.tile_pool(name="const", bufs=1) as cpool,
            tc.tile_pool(name="gather", bufs=2) as gpool,
            tc.tile_pool(name="chain", bufs=1) as chpool,
            tc.tile_pool(name="scratch", bufs=2) as spool,
            tc.tile_pool(name="accp", bufs=2) as apool,
        ):
            # single-buffered scratch: WAW-chains every DVE op in emission
            # order, so each op adds at most one new DMA-semaphore wait
            # (walrus rejects instructions with too many sync waits)
            prod = chpool.tile([P, EMB], bf16)
            idx_sb = cpool.tile([P, n_groups, NW, MAXW], i16)
            dense_sb = cpool.tile([P, n_tiles, DD + 1], f32)
            wvec_sb = cpool.tile([P, DD + 1], f32)
            out_all = cpool.tile([P, n_tiles], f32)
            nc.sync.dma_start(out=idx_sb[:], in_=idxs[:])
            nc.sync.dma_start(out=dense_sb[:], in_=dense_d[:])
            nc.sync.dma_start(out=wvec_sb[:], in_=wvec_d[:])

            for g_i in range(n_groups):
                gall = gpool.tile([P, F, n_g, ROW], bf16, tag="gall")
                for wi, (c0, ncw) in enumerate(WINDOWS):
                    nidx = ncw * n_g * P
                    nc.gpsimd.dma_gather(
                        gall[:, c0 : c0 + ncw, :, :].rearrange(
                            "p c n r -> p (c n) r"
                        ),
                        t2[c0 * v : (c0 + ncw) * v, :],
                        idx_sb[:, g_i, wi, : ncw * 8 * n_g],
                        nidx,
                        nidx,
                        ROW,
                        single_packet=SINGLE_PACKET,
                    )

                for n in range(n_g):
                    tt = g_i * n_g + n
                    acc2 = spool.tile([P, F + 1], f32, tag="acc2")
                    acc = apool.tile([P, F + 1], f32, tag="acc")
                    pre = apool.tile([P, 1], f32, tag="pre")

                    # cross: for each i, all pairs (i, j>i) fused into one
                    # multiply + per-partition reduce. Descending i so each
                    # op needs at most one gather window the engine hasn't
                    # already waited for.
                    for i in reversed(range(F - 1)):
                        cnt = F - 1 - i
                        x = gall[:, i, n, i * D : EMB].rearrange(
                            "p (j d) -> p j d", d=D
                        )
                        y = gall[:, i + 1 : F, n, i * D : (i + 1) * D]
                        nc.vector.scalar_tensor_tensor(
                            out=prod[:, : cnt * D].rearrange(
                                "p (j d) -> p j d", d=D
                            ),
                            in0=x,
                            scalar=1.0,
                            in1=y,
                            op0=mult,
                            op1=mult,
                            accum_out=acc[:, i : i + 1],
                        )

                    # linear sparse: sum the 20 w_sparse slots (chained via
                    # prod; op1=bypass makes it a pure sum of in0)
                    wsp_slice = gall[:, :, n, WSP : WSP + 1].rearrange(
                        "p c one -> p (c one)"
                    )
                    nc.vector.scalar_tensor_tensor(
                        out=prod[:, :F],
                        in0=wsp_slice,
                        scalar=1.0,
                        in1=wsp_slice,
                        op0=mult,
                        op1=mybir.AluOpType.bypass,
                        accum_out=acc[:, F - 1 : F],
                    )

                    # linear dense + bias
                    nc.vector.scalar_tensor_tensor(
                        out=prod[:, : DD + 1],
                        in0=dense_sb[:, tt, :],
                        scalar=1.0,
                        in1=wvec_sb[:],
                        op0=mult,
                        op1=mult,
                        accum_out=acc[:, F : F + 1],
                    )

                    # total + sigmoid
                    nc.scalar.activation(
                        out=acc2[:], in_=acc[:], func=copy_f, accum_out=pre[:]
                    )
                    nc.scalar.activation(
                        out=out_all[:, tt : tt + 1], in_=pre[:], func=sigm_f
                    )

            nc.sync.dma_start(out=out[:], in_=out_all[:])

    nc.compile()
    return nc


_PROGRAM_CACHE = {}


def _get_program():
    if "nc" not in _PROGRAM_CACHE:
        _PROGRAM_CACHE["nc"] = _build_program()
    return _PROGRAM_CACHE["nc"]


def make_idx_array(sparse_core, n_tiles=N_TILES, n_g=N_G, v=V):
    """sparse_core: [BPC, F] local ids (< V). Returns [P, n_groups, NW, MAXW] i16.

    dma_gather consumes idx element i from [partition i%16, col i//16],
    replicated 8x down the partitions. Within one (group, window) gather,
    i = (c_local * n_g + n) * 128 + p maps to out slot [p, c_local, n].
    """
    n_groups = n_tiles // n_g
    spc = sparse_core.reshape(P, n_tiles, F)  # [p, tt, c], sample s = p*n_tiles+tt
    arr = np.zeros((P, n_groups, NW, MAXW), dtype=np.int16)
    for g_i in range(n_groups):
        for wi, (c0, ncw) in enumerate(WINDOWS):
            # vals[c_local, n, p]
            vals = spc[:, g_i * n_g : (g_i + 1) * n_g, c0 : c0 + ncw].transpose(
                2, 1, 0
            ).astype(np.int64)
            vals = vals + (np.arange(ncw, dtype=np.int64) * v)[:, None, None]
            flat = vals.reshape(-1).astype(np.int16)  # i-ordered
            m = len(flat) // 16
            wrap = np.tile(flat.reshape(m, 16).T, (8, 1))  # [128, m]
            arr[:, g_i, wi, :m] = wrap
    return arr


def _prep_inputs(dense_input, sparse_input, tables, w_dense, w_sparse, bias):
    import ml_dtypes

    dense_input = np.asarray(dense_input, dtype=np.float32)
    sparse_input = np.asarray(sparse_input)
    tables = np.asarray(tables, dtype=np.float32)
    w_dense = np.asarray(w_dense, dtype=np.float32)
    w_sparse = np.asarray(w_sparse, dtype=np.float32)
    bias = np.asarray(bias, dtype=np.float32)

    # T2[g] = [tables[t, g, :] for t != g//V] ++ [w_sparse[g]] ++ pad, bf16
    t2 = np.zeros((VTOT, ROW), dtype=np.float32)
    for c in range(F):
        sl = slice(c * V, (c + 1) * V)
        sel = [t for t in range(F) if t != c]
        t2[sl, :EMB] = tables[sel, sl, :].transpose(1, 0, 2).reshape(V, EMB)
    t2[:, WSP] = w_sparse[:, 0]
    t2 = t2.astype(ml_dtypes.bfloat16)

    sparse_i = sparse_input.astype(np.int64).reshape(N_CORES, BPC, F)
    dense_aug = np.concatenate(
        [dense_input, np.ones((B, 1), dtype=np.float32)], axis=1
    ).reshape(N_CORES, P, N_TILES, DD + 1)
    waug = np.concatenate([w_dense[:, 0], bias]).astype(np.float32)
    wvec = np.tile(waug[None, :], (P, 1))

    in_maps = []
    for k in range(N_CORES):
        in_maps.append(
            {
                "t2": t2,
                "idxs": make_idx_array(sparse_i[k]),
                "dense": np.ascontiguousarray(dense_aug[k]),
                "wvec": wvec,
            }
        )
    return in_maps


def kernel(dense_input, sparse_input, tables, w_dense, w_sparse, bias, _trace=False):
    _, _, _, bass_utils = _import_concourse()

    nc = _get_program()
    in_maps = _prep_inputs(dense_input, sparse_input, tables, w_dense, w_sparse, bias)
    res = bass_utils.run_bass_kernel_spmd(
        nc, in_maps, core_ids=list(range(N_CORES)), trace=_trace
    )
    outs = [res.results[k]["out"].reshape(BPC) for k in range(N_CORES)]
    full = np.concatenate(outs).reshape(B, 1).astype(np.float32)
    if _trace:
        return full, res
    return full


# revision 17
# speedup vs baseline: 2.0032x; 2.0032x over previous
"""FFM layer (field-aware factorization machine) on 8 Trainium2 cores.

Strategy: data-parallel over batch (2048 samples/core). The embedding tables
are re-laid-out on the host into one row per global vocab id g (owned by
exactly one field c = g // V): the 19 *other* fields' embeddings for that id,
plus the w_sparse value, padded to 384 bf16 (768 B, the dma_gather 256 B
granularity). Each (sample, field) lookup is then one contiguous gather row.

The gather uses nc.gpsimd.dma_gather (int16 indices). Indices must fit int16,
so gathers address vocab windows of 3 fields (3*10000 < 32767), with
window-relative indices. One gather instruction covers 3 fields x 4
batch-tiles = 1536 rows.

The FFM cross term for pair (i, j>i) is dot(row_i[block j], row_j[block i]);
per 128-sample tile it is computed as 19 fused multiply + per-partition-reduce
vector ops (scalar_tensor_tensor with accum_out). w_sparse sum and the final
reduction + sigmoid run on the scalar engine; the dense linear part is one
more fused vector op against a replicated weight vector.
"""

import os
import sys

import numpy as np


def _import_concourse():
    try:
        import concourse  # noqa: F401
    except ImportError:
        for p in ("/opt/trn_rl_repo", "/root/.axon_site/_ro/trn_rl_repo"):
            if os.path.isdir(p) and p not in sys.path:
                sys.path.insert(0, p)
    import concourse.bass as bass  # noqa: F401
    import concourse.mybir as mybir  # noqa: F401
    import concourse.tile as tile  # noqa: F401
    from concourse import bass_utils  # noqa: F401

    return bass, mybir, tile, bass_utils


# Problem constants (hardcoded per contract)
F = 20          # sparse fields
V = 10000       # vocab per field
VTOT = F * V    # 200000
D = 16          # embed dim
B = 16384       # batch
DD = 13         # dense features
N_CORES = 8
P = 128         # SBUF partitions

BPC = B // N_CORES          # 2048 samples per core
N_TILES = BPC // P          # 16 tiles of 128 samples
ROW = 384                   # gather row (bf16): 19*16 emb + wsp + pad (768 B)
EMB = (F - 1) * D           # 304
WSP = EMB                   # w_sparse slot index
N_G = 4                     # batch-tiles per gather group
COLS_PER_WIN = 3            # fields per gather window (3*V < int16 max)
SINGLE_PACKET = False
N_QUEUES = 4

WINDOWS = [
    (c0, min(COLS_PER_WIN, F - c0)) for c0 in range(0, F, COLS_PER_WIN)
]
NW = len(WINDOWS)
MAXW = COLS_PER_WIN * 8 * N_G  # idx columns per (group, window) incl. 8x wrap


def _patch_queue_lanes():
    """Make Tile assign DMASW sem lanes per SWDGE queue (lane 2q/2q+1 for
    queue q) — the runtime locks each lane to one queue, but stock Tile
    round-robins lanes obliviously."""
    import concourse.tile_sem_assignment as tsa

    if getattr(tsa, "_ffm_queue_patch", False):
        return
    import concourse.mybir as mybir

    orig = tsa.TileClockTick._assign_tick

    def patched(self, inst):
        q = getattr(inst, "queue_num", None)
        if (
            q is not None
            and isinstance(inst, tsa.DMAInst)
            and inst.engine == mybir.EngineType.Pool
        ):
            state = getattr(self, "_ffm_perq", None)
            if state is None:
                state = {}
                self._ffm_perq = state
            self.next_sw_dma_idx = 2 * q + state.get(q, 0)
            orig(self, inst)
            state[q] = state.get(q, 0) ^ 1
            return
        orig(self, inst)

    tsa.TileClockTick._assign_tick = patched
    tsa._ffm_queue_patch = True


def _build_program(n_tiles=N_TILES, vtot=VTOT, n_g=N_G, for_sim=False):
    bass, mybir, tile, _ = _import_concourse()
    _patch_queue_lanes()

    v = vtot // F
    n_groups = n_tiles // n_g
    assert n_tiles % n_g == 0

    import concourse.bacc as bacc

    # Bacc (not plain Bass): its compile() runs generate_event_semaphores,
    # which splits multi-semaphore waits into InstEventSemaphore prefixes —
    # TRN2 instructions can carry only one inline wait — and inserts the
    # GPSIMD ucode library loads that dma_gather needs.
    nc = bacc.Bacc(None, target_bir_lowering=False, debug=for_sim,
                   num_swdge_queues=N_QUEUES)

    f32 = mybir.dt.float32
    bf16 = mybir.dt.bfloat16
    i16 = mybir.dt.int16
    mult = mybir.AluOpType.mult
    copy_f = mybir.ActivationFunctionType.Copy
    sigm_f = mybir.ActivationFunctionType.Sigmoid

    t2 = nc.dram_tensor("t2", [vtot, ROW], bf16, kind="ExternalInput")
    idxs = nc.dram_tensor("idxs", [P, n_groups, NW, MAXW], i16, kind="ExternalInput")
    dense_d = nc.dram_tensor("dense", [P, n_tiles, DD + 1], f32, kind="ExternalInput")
    wvec_d = nc.dram_tensor("wvec", [P, DD + 1], f32, kind="ExternalInput")
    out = nc.dram_tensor("out", [P, n_tiles], f32, kind="ExternalOutput")

    with tile.TileContext(nc) as tc:
        with (
            tc.tile_pool(name="const", bufs=1) as cpool,
            tc.tile_pool(name="gather", bufs=2) as gpool,
            tc.tile_pool(name="chain", bufs=1) as chpool,
            tc.tile_pool(name="scratch", bufs=2) as spool,
            tc.tile_pool(name="accp", bufs=2) as apool,
        ):
            # single-buffered scratch: WAW-chains every DVE op in emission
            # order, so each op adds at most one new DMA-semaphore wait
            # (walrus rejects instructions with too many sync waits)
            prod = chpool.tile([P, EMB], bf16)
            idx_sb = cpool.tile([P, n_groups, NW, MAXW], i16)
            dense_sb = cpool.tile([P, n_tiles, DD + 1], f32)
            wvec_sb = cpool.tile([P, DD + 1], f32)
            out_all = cpool.tile([P, n_tiles], f32)
            nc.sync.dma_start(out=idx_sb[:], in_=idxs[:])
            nc.sync.dma_start(out=dense_sb[:], in_=dense_d[:])
            nc.sync.dma_start(out=wvec_sb[:], in_=wvec_d[:])

            for g_i in range(n_groups):
                gall = gpool.tile([P, F, n_g, ROW], bf16, tag="gall")
                for wi, (c0, ncw) in enumerate(WINDOWS):
                    nidx = ncw * n_g * P
                    nc.gpsimd.dma_gather(
                        gall[:, c0 : c0 + ncw, :, :].rearrange(
                            "p c n r -> p (c n) r"
                        ),
                        t2[c0 * v : (c0 + ncw) * v, :],
                        idx_sb[:, g_i, wi, : ncw * 8 * n_g],
                        nidx,
                        nidx,
                        ROW,
                        single_packet=SINGLE_PACKET,
                        queue_num=wi % N_QUEUES,
                    )

                for n in range(n_g):
                    tt = g_i * n_g + n
                    acc2 = spool.tile([P, F + 1], f32, tag="acc2")
                    acc = apool.tile([P, F + 1], f32, tag="acc")
                    pre = apool.tile([P, 1], f32, tag="pre")

                    # cross: for each i, all pairs (i, j>i) fused into one
                    # multiply + per-partition reduce. Descending i so each
                    # op needs at most one gather window the engine hasn't
                    # already waited for.
                    for i in reversed(range(F - 1)):
                        cnt = F - 1 - i
                        x = gall[:, i, n, i * D : EMB].rearrange(
                            "p (j d) -> p j d", d=D
                        )
                        y = gall[:, i + 1 : F, n, i * D : (i + 1) * D]
                        nc.vector.scalar_tensor_tensor(
                            out=prod[:, : cnt * D].rearrange(
                                "p (j d) -> p j d", d=D
                            ),
                            in0=x,
                            scalar=1.0,
                            in1=y,
                            op0=mult,
                            op1=mult,
                            accum_out=acc[:, i : i + 1],
                        )

                    # linear sparse: sum the 20 w_sparse slots (chained via
                    # prod; op1=bypass makes it a pure sum of in0)
                    wsp_slice = gall[:, :, n, WSP : WSP + 1].rearrange(
                        "p c one -> p (c one)"
                    )
                    nc.vector.scalar_tensor_tensor(
                        out=prod[:, :F],
                        in0=wsp_slice,
                        scalar=1.0,
                        in1=wsp_slice,
                        op0=mult,
                        op1=mybir.AluOpType.bypass,
                        accum_out=acc[:, F - 1 : F],
                    )

                    # linear dense + bias
                    nc.vector.scalar_tensor_tensor(
                        out=prod[:, : DD + 1],
                        in0=dense_sb[:, tt, :],
                        scalar=1.0,
                        in1=wvec_sb[:],
                        op0=mult,
                        op1=mult,
                        accum_out=acc[:, F : F + 1],
                    )

                    # total + sigmoid
                    nc.scalar.activation(
                        out=acc2[:], in_=acc[:], func=copy_f, accum_out=pre[:]
                    )
                    nc.scalar.activation(
                        out=out_all[:, tt : tt + 1], in_=pre[:], func=sigm_f
                    )

            nc.sync.dma_start(out=out[:], in_=out_all[:])

    nc.compile()
    return nc


_PROGRAM_CACHE = {}


def _get_program():
    if "nc" not in _PROGRAM_CACHE:
        _PROGRAM_CACHE["nc"] = _build_program()
    return _PROGRAM_CACHE["nc"]


def make_idx_array(sparse_core, n_tiles=N_TILES, n_g=N_G, v=V):
    """sparse_core: [BPC, F] local ids (< V). Returns [P, n_groups, NW, MAXW] i16.

    dma_gather consumes idx element i from [partition i%16, col i//16],
    replicated 8x down the partitions. Within one (group, window) gather,
    i = (c_local * n_g + n) * 128 + p maps to out slot [p, c_local, n].
    """
    n_groups = n_tiles // n_g
    spc = sparse_core.reshape(P, n_tiles, F)  # [p, tt, c], sample s = p*n_tiles+tt
    arr = np.zeros((P, n_groups, NW, MAXW), dtype=np.int16)
    for g_i in range(n_groups):
        for wi, (c0, ncw) in enumerate(WINDOWS):
            # vals[c_local, n, p]
            vals = spc[:, g_i * n_g : (g_i + 1) * n_g, c0 : c0 + ncw].transpose(
                2, 1, 0
            ).astype(np.int64)
            vals = vals + (np.arange(ncw, dtype=np.int64) * v)[:, None, None]
            flat = vals.reshape(-1).astype(np.int16)  # i-ordered
            m = len(flat) // 16
            wrap = np.tile(flat.reshape(m, 16).T, (8, 1))  # [128, m]
            arr[:, g_i, wi, :m] = wrap
    return arr


def _prep_inputs(dense_input, sparse_input, tables, w_dense, w_sparse, bias):
    import ml_dtypes

    dense_input = np.asarray(dense_input, dtype=np.float32)
    sparse_input = np.asarray(sparse_input)
    tables = np.asarray(tables, dtype=np.float32)
    w_dense = np.asarray(w_dense, dtype=np.float32)
    w_sparse = np.asarray(w_sparse, dtype=np.float32)
    bias = np.asarray(bias, dtype=np.float32)

    # T2[g] = [tables[t, g, :] for t != g//V] ++ [w_sparse[g]] ++ pad, bf16
    t2 = np.zeros((VTOT, ROW), dtype=np.float32)
    for c in range(F):
        sl = slice(c * V, (c + 1) * V)
        sel = [t for t in range(F) if t != c]
        t2[sl, :EMB] = tables[sel, sl, :].transpose(1, 0, 2).reshape(V, EMB)
    t2[:, WSP] = w_sparse[:, 0]
    t2 = t2.astype(ml_dtypes.bfloat16)

    sparse_i = sparse_input.astype(np.int64).reshape(N_CORES, BPC, F)
    dense_aug = np.concatenate(
        [dense_input, np.ones((B, 1), dtype=np.float32)], axis=1
    ).reshape(N_CORES, P, N_TILES, DD + 1)
    waug = np.concatenate([w_dense[:, 0], bias]).astype(np.float32)
    wvec = np.tile(waug[None, :], (P, 1))

    in_maps = []
    for k in range(N_CORES):
        in_maps.append(
            {
                "t2": t2,
                "idxs": make_idx_array(sparse_i[k]),
                "dense": np.ascontiguousarray(dense_aug[k]),
                "wvec": wvec,
            }
        )
    return in_maps


def kernel(dense_input, sparse_input, tables, w_dense, w_sparse, bias, _trace=False):
    _, _, _, bass_utils = _import_concourse()

    nc = _get_program()
    in_maps = _prep_inputs(dense_input, sparse_input, tables, w_dense, w_sparse, bias)
    res = bass_utils.run_bass_kernel_spmd(
        nc, in_maps, core_ids=list(range(N_CORES)), trace=_trace
    )
    outs = [res.results[k]["out"].reshape(BPC) for k in range(N_CORES)]
    full = np.concatenate(outs).reshape(B, 1).astype(np.float32)
    if _trace:
        return full, res
    return full


# revision 19
# speedup vs baseline: 2.1777x; 1.0871x over previous
"""FFM layer (field-aware factorization machine) on 8 Trainium2 cores.

Strategy: data-parallel over batch (2048 samples/core). The embedding tables
are re-laid-out on the host into one row per global vocab id g (owned by
exactly one field c = g // V): the 19 *other* fields' embeddings for that id,
plus the w_sparse value, padded to 384 bf16 (768 B, the dma_gather 256 B
granularity). Each (sample, field) lookup is then one contiguous gather row.

The gather uses nc.gpsimd.dma_gather (int16 indices). Indices must fit int16,
so gathers address vocab windows of 3 fields (3*10000 < 32767), with
window-relative indices. One gather instruction covers 3 fields x 4
batch-tiles = 1536 rows.

The FFM cross term for pair (i, j>i) is dot(row_i[block j], row_j[block i]);
per 128-sample tile it is computed as 19 fused multiply + per-partition-reduce
vector ops (scalar_tensor_tensor with accum_out). w_sparse sum and the final
reduction + sigmoid run on the scalar engine; the dense linear part is one
more fused vector op against a replicated weight vector.
"""

import os
import sys

import numpy as np


def _import_concourse():
    try:
        import concourse  # noqa: F401
    except ImportError:
        for p in ("/opt/trn_rl_repo", "/root/.axon_site/_ro/trn_rl_repo"):
            if os.path.isdir(p) and p not in sys.path:
                sys.path.insert(0, p)
    import concourse.bass as bass  # noqa: F401
    import concourse.mybir as mybir  # noqa: F401
    import concourse.tile as tile  # noqa: F401
    from concourse import bass_utils  # noqa: F401

    return bass, mybir, tile, bass_utils


# Problem constants (hardcoded per contract)
F = 20          # sparse fields
V = 10000       # vocab per field
VTOT = F * V    # 200000
D = 16          # embed dim
B = 16384       # batch
DD = 13         # dense features
N_CORES = 8
P = 128         # SBUF partitions

BPC = B // N_CORES          # 2048 samples per core
N_TILES = BPC // P          # 16 tiles of 128 samples
ROW = 384                   # gather row (bf16): 19*16 emb + wsp + pad (768 B)
EMB = (F - 1) * D           # 304
WSP = EMB                   # w_sparse slot index
N_G = 4                     # batch-tiles per gather group
COLS_PER_WIN = 3            # fields per gather window (3*V < int16 max)
SINGLE_PACKET = False
N_QUEUES = 4

WINDOWS = [
    (c0, min(COLS_PER_WIN, F - c0)) for c0 in range(0, F, COLS_PER_WIN)
]
NW = len(WINDOWS)
MAXW = COLS_PER_WIN * 8 * N_G  # idx columns per (group, window) incl. 8x wrap


def _patch_queue_lanes():
    """Make Tile assign DMASW sem lanes per SWDGE queue (lane 2q/2q+1 for
    queue q) — the runtime locks each lane to one queue, but stock Tile
    round-robins lanes obliviously."""
    import concourse.tile_sem_assignment as tsa

    if getattr(tsa, "_ffm_queue_patch", False):
        return
    import concourse.mybir as mybir

    orig = tsa.TileClockTick._assign_tick

    def patched(self, inst):
        q = getattr(inst, "queue_num", None)
        if (
            q is not None
            and isinstance(inst, tsa.DMAInst)
            and inst.engine == mybir.EngineType.Pool
        ):
            state = getattr(self, "_ffm_perq", None)
            if state is None:
                state = {}
                self._ffm_perq = state
            self.next_sw_dma_idx = 2 * q + state.get(q, 0)
            orig(self, inst)
            state[q] = state.get(q, 0) ^ 1
            return
        orig(self, inst)

    tsa.TileClockTick._assign_tick = patched
    tsa._ffm_queue_patch = True


def _build_program(n_tiles=N_TILES, vtot=VTOT, n_g=N_G, for_sim=False):
    bass, mybir, tile, _ = _import_concourse()
    _patch_queue_lanes()

    v = vtot // F
    n_groups = n_tiles // n_g
    assert n_tiles % n_g == 0

    import concourse.bacc as bacc

    # Bacc (not plain Bass): its compile() runs generate_event_semaphores,
    # which splits multi-semaphore waits into InstEventSemaphore prefixes —
    # TRN2 instructions can carry only one inline wait — and inserts the
    # GPSIMD ucode library loads that dma_gather needs.
    nc = bacc.Bacc(None, target_bir_lowering=False, debug=for_sim,
                   num_swdge_queues=N_QUEUES)

    f32 = mybir.dt.float32
    bf16 = mybir.dt.bfloat16
    i16 = mybir.dt.int16
    mult = mybir.AluOpType.mult
    copy_f = mybir.ActivationFunctionType.Copy
    sigm_f = mybir.ActivationFunctionType.Sigmoid

    t2 = nc.dram_tensor("t2", [vtot, ROW], bf16, kind="ExternalInput")
    idxs = nc.dram_tensor("idxs", [P, n_groups, NW, MAXW], i16, kind="ExternalInput")
    dense_d = nc.dram_tensor("dense", [DD + 1, n_tiles, P], f32, kind="ExternalInput")
    wvec_d = nc.dram_tensor("wvec", [DD + 1, 1], f32, kind="ExternalInput")
    out = nc.dram_tensor("out", [P, n_tiles], f32, kind="ExternalOutput")

    with tile.TileContext(nc) as tc:
        with (
            tc.tile_pool(name="const", bufs=1) as cpool,
            tc.tile_pool(name="gather", bufs=2) as gpool,
            tc.tile_pool(name="chain", bufs=1) as chpool,
            tc.tile_pool(name="scratch", bufs=2) as spool,
            tc.tile_pool(name="accp", bufs=2) as apool,
            tc.tile_pool(name="psum", bufs=2, space="PSUM") as pspool,
        ):
            # single-buffered scratch: WAW-chains every DVE op in emission
            # order, so each op adds at most one new DMA-semaphore wait
            # (walrus rejects instructions with too many sync waits)
            prod = chpool.tile([P, EMB], bf16)
            idx_sb = cpool.tile([P, n_groups, NW, MAXW], i16)
            dense_sb = cpool.tile([DD + 1, n_tiles, P], f32)
            wvec_sb = cpool.tile([DD + 1, 1], f32)
            out_all = cpool.tile([P, n_tiles], f32)
            nc.sync.dma_start(out=idx_sb[:], in_=idxs[:])
            nc.sync.dma_start(out=dense_sb[:], in_=dense_d[:])
            nc.sync.dma_start(out=wvec_sb[:], in_=wvec_d[:])

            gather_seq = 0
            for g_i in range(n_groups):
                gall = gpool.tile([P, F, n_g, ROW], bf16, tag="gall")
                # reversed window order: the descending-i compute consumes
                # windows last-to-first, so emitting w6 first lets compute
                # start after the first gather of the group lands
                for wi, (c0, ncw) in reversed(list(enumerate(WINDOWS))):
                    nidx = ncw * n_g * P
                    nc.gpsimd.dma_gather(
                        gall[:, c0 : c0 + ncw, :, :].rearrange(
                            "p c n r -> p (c n) r"
                        ),
                        t2[c0 * v : (c0 + ncw) * v, :],
                        idx_sb[:, g_i, wi, : ncw * 8 * n_g],
                        nidx,
                        nidx,
                        ROW,
                        single_packet=SINGLE_PACKET,
                        queue_num=gather_seq % N_QUEUES,
                    )
                    gather_seq += 1

                for n in range(n_g):
                    tt = g_i * n_g + n
                    acc2 = spool.tile([P, F + 1], f32, tag="acc2")
                    acc = apool.tile([P, F + 1], f32, tag="acc")
                    pre = apool.tile([P, 1], f32, tag="pre")

                    # cross: for each i, all pairs (i, j>i) fused into one
                    # multiply + per-partition reduce. Descending i so each
                    # op needs at most one gather window the engine hasn't
                    # already waited for.
                    for i in reversed(range(F - 1)):
                        cnt = F - 1 - i
                        x = gall[:, i, n, i * D : EMB].rearrange(
                            "p (j d) -> p j d", d=D
                        )
                        y = gall[:, i + 1 : F, n, i * D : (i + 1) * D]
                        nc.vector.scalar_tensor_tensor(
                            out=prod[:, : cnt * D].rearrange(
                                "p (j d) -> p j d", d=D
                            ),
                            in0=x,
                            scalar=1.0,
                            in1=y,
                            op0=mult,
                            op1=mult,
                            accum_out=acc[:, i : i + 1],
                        )

                    # linear sparse: sum the 20 w_sparse slots (on ACT,
                    # which has slack)
                    wcp = spool.tile([P, F], bf16, tag="wcp")
                    nc.scalar.activation(
                        out=wcp[:],
                        in_=gall[:, :, n, WSP : WSP + 1].rearrange(
                            "p c one -> p (c one)"
                        ),
                        func=copy_f,
                        accum_out=acc[:, F - 1 : F],
                    )

                    # linear dense + bias on the (idle) tensor engine
                    ps = pspool.tile([P, 1], f32)
                    nc.tensor.matmul(
                        out=ps[:],
                        lhsT=dense_sb[:, tt, :],
                        rhs=wvec_sb[:, :1],
                        start=True,
                        stop=True,
                    )
                    nc.scalar.copy(out=acc[:, F : F + 1], in_=ps[:])

                    # total + sigmoid
                    nc.scalar.activation(
                        out=acc2[:], in_=acc[:], func=copy_f, accum_out=pre[:]
                    )
                    nc.scalar.activation(
                        out=out_all[:, tt : tt + 1], in_=pre[:], func=sigm_f
                    )

            nc.sync.dma_start(out=out[:], in_=out_all[:])

    nc.compile()
    return nc


_PROGRAM_CACHE = {}


def _get_program():
    if "nc" not in _PROGRAM_CACHE:
        _PROGRAM_CACHE["nc"] = _build_program()
    return _PROGRAM_CACHE["nc"]


def make_idx_array(sparse_core, n_tiles=N_TILES, n_g=N_G, v=V):
    """sparse_core: [BPC, F] local ids (< V). Returns [P, n_groups, NW, MAXW] i16.

    dma_gather consumes idx element i from [partition i%16, col i//16],
    replicated 8x down the partitions. Within one (group, window) gather,
    i = (c_local * n_g + n) * 128 + p maps to out slot [p, c_local, n].
    """
    n_groups = n_tiles // n_g
    spc = sparse_core.reshape(P, n_tiles, F)  # [p, tt, c], sample s = p*n_tiles+tt
    arr = np.zeros((P, n_groups, NW, MAXW), dtype=np.int16)
    for g_i in range(n_groups):
        for wi, (c0, ncw) in enumerate(WINDOWS):
            # vals[c_local, n, p]
            vals = spc[:, g_i * n_g : (g_i + 1) * n_g, c0 : c0 + ncw].transpose(
                2, 1, 0
            ).astype(np.int64)
            vals = vals + (np.arange(ncw, dtype=np.int64) * v)[:, None, None]
            flat = vals.reshape(-1).astype(np.int16)  # i-ordered
            m = len(flat) // 16
            wrap = np.tile(flat.reshape(m, 16).T, (8, 1))  # [128, m]
            arr[:, g_i, wi, :m] = wrap
    return arr


def _prep_inputs(dense_input, sparse_input, tables, w_dense, w_sparse, bias):
    import ml_dtypes

    dense_input = np.asarray(dense_input, dtype=np.float32)
    sparse_input = np.asarray(sparse_input)
    tables = np.asarray(tables, dtype=np.float32)
    w_dense = np.asarray(w_dense, dtype=np.float32)
    w_sparse = np.asarray(w_sparse, dtype=np.float32)
    bias = np.asarray(bias, dtype=np.float32)

    # T2[g] = [tables[t, g, :] for t != g//V] ++ [w_sparse[g]] ++ pad, bf16
    t2 = np.zeros((VTOT, ROW), dtype=np.float32)
    for c in range(F):
        sl = slice(c * V, (c + 1) * V)
        sel = [t for t in range(F) if t != c]
        t2[sl, :EMB] = tables[sel, sl, :].transpose(1, 0, 2).reshape(V, EMB)
    t2[:, WSP] = w_sparse[:, 0]
    t2 = t2.astype(ml_dtypes.bfloat16)

    sparse_i = sparse_input.astype(np.int64).reshape(N_CORES, BPC, F)
    dense_aug = np.concatenate(
        [dense_input, np.ones((B, 1), dtype=np.float32)], axis=1
    ).reshape(N_CORES, P, N_TILES, DD + 1)
    waug = np.concatenate([w_dense[:, 0], bias]).astype(np.float32)
    wvec = waug.reshape(DD + 1, 1)

    in_maps = []
    for k in range(N_CORES):
        in_maps.append(
            {
                "t2": t2,
                "idxs": make_idx_array(sparse_i[k]),
                "dense": np.ascontiguousarray(dense_aug[k].transpose(2, 1, 0)),
                "wvec": wvec,
            }
        )
    return in_maps


def kernel(dense_input, sparse_input, tables, w_dense, w_sparse, bias, _trace=False):
    _, _, _, bass_utils = _import_concourse()

    nc = _get_program()
    in_maps = _prep_inputs(dense_input, sparse_input, tables, w_dense, w_sparse, bias)
    res = bass_utils.run_bass_kernel_spmd(
        nc, in_maps, core_ids=list(range(N_CORES)), trace=_trace
    )
    outs = [res.results[k]["out"].reshape(BPC) for k in range(N_CORES)]
    full = np.concatenate(outs).reshape(B, 1).astype(np.float32)
    if _trace:
        return full, res
    return full


# revision 20
# speedup vs baseline: 2.3558x; 1.0818x over previous
"""FFM layer (field-aware factorization machine) on 8 Trainium2 cores.

Strategy: data-parallel over batch (2048 samples/core). The embedding tables
are re-laid-out on the host into one row per global vocab id g (owned by
exactly one field c = g // V): the 19 *other* fields' embeddings for that id,
plus the w_sparse value, padded to 384 bf16 (768 B, the dma_gather 256 B
granularity). Each (sample, field) lookup is then one contiguous gather row.

The gather uses nc.gpsimd.dma_gather (int16 indices). Indices must fit int16,
so gathers address vocab windows of 3 fields (3*10000 < 32767), with
window-relative indices. One gather instruction covers 3 fields x 4
batch-tiles = 1536 rows.

The FFM cross term for pair (i, j>i) is dot(row_i[block j], row_j[block i]);
per 128-sample tile it is computed as 19 fused multiply + per-partition-reduce
vector ops (scalar_tensor_tensor with accum_out). w_sparse sum and the final
reduction + sigmoid run on the scalar engine; the dense linear part is one
more fused vector op against a replicated weight vector.
"""

import os
import sys

import numpy as np


def _import_concourse():
    try:
        import concourse  # noqa: F401
    except ImportError:
        for p in ("/opt/trn_rl_repo", "/root/.axon_site/_ro/trn_rl_repo"):
            if os.path.isdir(p) and p not in sys.path:
                sys.path.insert(0, p)
    import concourse.bass as bass  # noqa: F401
    import concourse.mybir as mybir  # noqa: F401
    import concourse.tile as tile  # noqa: F401
    from concourse import bass_utils  # noqa: F401

    return bass, mybir, tile, bass_utils


# Problem constants (hardcoded per contract)
F = 20          # sparse fields
V = 10000       # vocab per field
VTOT = F * V    # 200000
D = 16          # embed dim
B = 16384       # batch
DD = 13         # dense features
N_CORES = 8
P = 128         # SBUF partitions

BPC = B // N_CORES          # 2048 samples per core
N_TILES = BPC // P          # 16 tiles of 128 samples
ROW = 384                   # gather row (bf16): 19*16 emb + wsp + pad (768 B)
EMB = (F - 1) * D           # 304
WSP = EMB                   # w_sparse slot index
NPAIR_ELEMS = (F * (F - 1) // 2) * D  # 3040 pair-product elements per sample
N_G = 4                     # batch-tiles per gather group
COLS_PER_WIN = 3            # fields per gather window (3*V < int16 max)
SINGLE_PACKET = False
N_QUEUES = 4

WINDOWS = [
    (c0, min(COLS_PER_WIN, F - c0)) for c0 in range(0, F, COLS_PER_WIN)
]
NW = len(WINDOWS)
MAXW = COLS_PER_WIN * 8 * N_G  # idx columns per (group, window) incl. 8x wrap


def _patch_queue_lanes():
    """Make Tile assign DMASW sem lanes per SWDGE queue (lane 2q/2q+1 for
    queue q) — the runtime locks each lane to one queue, but stock Tile
    round-robins lanes obliviously."""
    import concourse.tile_sem_assignment as tsa

    if getattr(tsa, "_ffm_queue_patch", False):
        return
    import concourse.mybir as mybir

    orig = tsa.TileClockTick._assign_tick

    def patched(self, inst):
        q = getattr(inst, "queue_num", None)
        if (
            q is not None
            and isinstance(inst, tsa.DMAInst)
            and inst.engine == mybir.EngineType.Pool
        ):
            state = getattr(self, "_ffm_perq", None)
            if state is None:
                state = {}
                self._ffm_perq = state
            self.next_sw_dma_idx = 2 * q + state.get(q, 0)
            orig(self, inst)
            state[q] = state.get(q, 0) ^ 1
            return
        orig(self, inst)

    tsa.TileClockTick._assign_tick = patched
    tsa._ffm_queue_patch = True


def _build_program(n_tiles=N_TILES, vtot=VTOT, n_g=N_G, for_sim=False):
    bass, mybir, tile, _ = _import_concourse()
    _patch_queue_lanes()

    v = vtot // F
    n_groups = n_tiles // n_g
    assert n_tiles % n_g == 0

    import concourse.bacc as bacc

    # Bacc (not plain Bass): its compile() runs generate_event_semaphores,
    # which splits multi-semaphore waits into InstEventSemaphore prefixes —
    # TRN2 instructions can carry only one inline wait — and inserts the
    # GPSIMD ucode library loads that dma_gather needs.
    nc = bacc.Bacc(None, target_bir_lowering=False, debug=for_sim,
                   num_swdge_queues=N_QUEUES)

    f32 = mybir.dt.float32
    bf16 = mybir.dt.bfloat16
    i16 = mybir.dt.int16
    mult = mybir.AluOpType.mult
    copy_f = mybir.ActivationFunctionType.Copy
    sigm_f = mybir.ActivationFunctionType.Sigmoid

    t2 = nc.dram_tensor("t2", [vtot, ROW], bf16, kind="ExternalInput")
    idxs = nc.dram_tensor("idxs", [P, n_groups, NW, MAXW], i16, kind="ExternalInput")
    dense_d = nc.dram_tensor("dense", [DD + 1, n_tiles, P], f32, kind="ExternalInput")
    wvec_d = nc.dram_tensor("wvec", [DD + 1, 1], f32, kind="ExternalInput")
    out = nc.dram_tensor("out", [P, n_tiles], f32, kind="ExternalOutput")

    with tile.TileContext(nc) as tc:
        with (
            tc.tile_pool(name="const", bufs=1) as cpool,
            tc.tile_pool(name="gather", bufs=2) as gpool,
            tc.tile_pool(name="scratch", bufs=2) as spool,
            tc.tile_pool(name="accp", bufs=4) as apool,
            tc.tile_pool(name="psum", bufs=2, space="PSUM") as pspool,
        ):
            idx_sb = cpool.tile([P, n_groups, NW, MAXW], i16)
            dense_sb = cpool.tile([DD + 1, n_tiles, P], f32)
            wvec_sb = cpool.tile([DD + 1, 1], f32)
            out_all = cpool.tile([P, n_tiles], f32)
            nc.sync.dma_start(out=idx_sb[:], in_=idxs[:])
            nc.sync.dma_start(out=dense_sb[:], in_=dense_d[:])
            nc.sync.dma_start(out=wvec_sb[:], in_=wvec_d[:])

            gather_seq = 0
            for g_i in range(n_groups):
                gall = gpool.tile([P, F, n_g, ROW], bf16, tag="gall")
                # reversed window order: the descending-i compute consumes
                # windows last-to-first, so emitting w6 first lets compute
                # start after the first gather of the group lands
                for wi, (c0, ncw) in reversed(list(enumerate(WINDOWS))):
                    nidx = ncw * n_g * P
                    nc.gpsimd.dma_gather(
                        gall[:, c0 : c0 + ncw, :, :].rearrange(
                            "p c n r -> p (c n) r"
                        ),
                        t2[c0 * v : (c0 + ncw) * v, :],
                        idx_sb[:, g_i, wi, : ncw * 8 * n_g],
                        nidx,
                        nidx,
                        ROW,
                        single_packet=SINGLE_PACKET,
                        queue_num=gather_seq % N_QUEUES,
                    )
                    gather_seq += 1

                for n in range(n_g):
                    tt = g_i * n_g + n
                    prod = spool.tile([P, NPAIR_ELEMS], bf16, tag="prod")
                    pcp = spool.tile([P, NPAIR_ELEMS], bf16, tag="pcp")
                    acc2 = spool.tile([P, 3], f32, tag="acc2")
                    acc = apool.tile([P, 3], f32, tag="acc")
                    pre = apool.tile([P, 1], f32, tag="pre")

                    # cross: per i, the pair products (i, j>i) as one plain
                    # tensor_tensor multiply into a slice of prod (eligible
                    # for the DVE 16-bit 2x perf mode). Descending i so each
                    # op needs at most one gather window the engine hasn't
                    # already waited for.
                    off = 0
                    for i in reversed(range(F - 1)):
                        cnt = F - 1 - i
                        x = gall[:, i, n, i * D : EMB].rearrange(
                            "p (j d) -> p j d", d=D
                        )
                        y = gall[:, i + 1 : F, n, i * D : (i + 1) * D]
                        nc.vector.tensor_tensor(
                            out=prod[:, off : off + cnt * D].rearrange(
                                "p (j d) -> p j d", d=D
                            ),
                            in0=x,
                            in1=y,
                            op=mult,
                        )
                        off += cnt * D

                    # cross sum: one ACT accumulate over all pair products
                    nc.scalar.activation(
                        out=pcp[:],
                        in_=prod[:],
                        func=copy_f,
                        accum_out=acc[:, 0:1],
                    )

                    # linear sparse: sum the 20 w_sparse slots (on ACT)
                    wcp = spool.tile([P, F], bf16, tag="wcp")
                    nc.scalar.activation(
                        out=wcp[:],
                        in_=gall[:, :, n, WSP : WSP + 1].rearrange(
                            "p c one -> p (c one)"
                        ),
                        func=copy_f,
                        accum_out=acc[:, 1:2],
                    )

                    # linear dense + bias on the (idle) tensor engine
                    ps = pspool.tile([P, 1], f32)
                    nc.tensor.matmul(
                        out=ps[:],
                        lhsT=dense_sb[:, tt, :],
                        rhs=wvec_sb[:, :1],
                        start=True,
                        stop=True,
                    )
                    nc.scalar.copy(out=acc[:, 2:3], in_=ps[:])

                    # total + sigmoid
                    nc.scalar.activation(
                        out=acc2[:], in_=acc[:], func=copy_f, accum_out=pre[:]
                    )
                    nc.scalar.activation(
                        out=out_all[:, tt : tt + 1], in_=pre[:], func=sigm_f
                    )

            nc.sync.dma_start(out=out[:], in_=out_all[:])

    nc.compile()
    return nc


_PROGRAM_CACHE = {}


def _get_program():
    if "nc" not in _PROGRAM_CACHE:
        _PROGRAM_CACHE["nc"] = _build_program()
    return _PROGRAM_CACHE["nc"]


def make_idx_array(sparse_core, n_tiles=N_TILES, n_g=N_G, v=V):
    """sparse_core: [BPC, F] local ids (< V). Returns [P, n_groups, NW, MAXW] i16.

    dma_gather consumes idx element i from [partition i%16, col i//16],
    replicated 8x down the partitions. Within one (group, window) gather,
    i = (c_local * n_g + n) * 128 + p maps to out slot [p, c_local, n].
    """
    n_groups = n_tiles // n_g
    spc = sparse_core.reshape(P, n_tiles, F)  # [p, tt, c], sample s = p*n_tiles+tt
    arr = np.zeros((P, n_groups, NW, MAXW), dtype=np.int16)
    for g_i in range(n_groups):
        for wi, (c0, ncw) in enumerate(WINDOWS):
            # vals[c_local, n, p]
            vals = spc[:, g_i * n_g : (g_i + 1) * n_g, c0 : c0 + ncw].transpose(
                2, 1, 0
            ).astype(np.int64)
            vals = vals + (np.arange(ncw, dtype=np.int64) * v)[:, None, None]
            flat = vals.reshape(-1).astype(np.int16)  # i-ordered
            m = len(flat) // 16
            wrap = np.tile(flat.reshape(m, 16).T, (8, 1))  # [128, m]
            arr[:, g_i, wi, :m] = wrap
    return arr


def _prep_inputs(dense_input, sparse_input, tables, w_dense, w_sparse, bias):
    import ml_dtypes

    dense_input = np.asarray(dense_input, dtype=np.float32)
    sparse_input = np.asarray(sparse_input)
    tables = np.asarray(tables, dtype=np.float32)
    w_dense = np.asarray(w_dense, dtype=np.float32)
    w_sparse = np.asarray(w_sparse, dtype=np.float32)
    bias = np.asarray(bias, dtype=np.float32)

    # T2[g] = [tables[t, g, :] for t != g//V] ++ [w_sparse[g]] ++ pad, bf16
    t2 = np.zeros((VTOT, ROW), dtype=np.float32)
    for c in range(F):
        sl = slice(c * V, (c + 1) * V)
        sel = [t for t in range(F) if t != c]
        t2[sl, :EMB] = tables[sel, sl, :].transpose(1, 0, 2).reshape(V, EMB)
    t2[:, WSP] = w_sparse[:, 0]
    t2 = t2.astype(ml_dtypes.bfloat16)

    sparse_i = sparse_input.astype(np.int64).reshape(N_CORES, BPC, F)
    dense_aug = np.concatenate(
        [dense_input, np.ones((B, 1), dtype=np.float32)], axis=1
    ).reshape(N_CORES, P, N_TILES, DD + 1)
    waug = np.concatenate([w_dense[:, 0], bias]).astype(np.float32)
    wvec = waug.reshape(DD + 1, 1)

    in_maps = []
    for k in range(N_CORES):
        in_maps.append(
            {
                "t2": t2,
                "idxs": make_idx_array(sparse_i[k]),
                "dense": np.ascontiguousarray(dense_aug[k].transpose(2, 1, 0)),
                "wvec": wvec,
            }
        )
    return in_maps


def kernel(dense_input, sparse_input, tables, w_dense, w_sparse, bias, _trace=False):
    _, _, _, bass_utils = _import_concourse()

    nc = _get_program()
    in_maps = _prep_inputs(dense_input, sparse_input, tables, w_dense, w_sparse, bias)
    res = bass_utils.run_bass_kernel_spmd(
        nc, in_maps, core_ids=list(range(N_CORES)), trace=_trace
    )
    outs = [res.results[k]["out"].reshape(BPC) for k in range(N_CORES)]
    full = np.concatenate(outs).reshape(B, 1).astype(np.float32)
    if _trace:
        return full, res
    return full


# revision 21
# speedup vs baseline: 2.5609x; 1.0871x over previous
"""FFM layer (field-aware factorization machine) on 8 Trainium2 cores.

Strategy: data-parallel over batch (2048 samples/core). The embedding tables
are re-laid-out on the host into one row per global vocab id g (owned by
exactly one field c = g // V): the 19 *other* fields' embeddings for that id,
plus the w_sparse value, padded to 384 bf16 (768 B, the dma_gather 256 B
granularity). Each (sample, field) lookup is then one contiguous gather row.

The gather uses nc.gpsimd.dma_gather (int16 indices). Indices must fit int16,
so gathers address vocab windows of 3 fields (3*10000 < 32767), with
window-relative indices. One gather instruction covers 3 fields x 4
batch-tiles = 1536 rows.

The FFM cross term for pair (i, j>i) is dot(row_i[block j], row_j[block i]);
per 128-sample tile it is computed as 19 fused multiply + per-partition-reduce
vector ops (scalar_tensor_tensor with accum_out). w_sparse sum and the final
reduction + sigmoid run on the scalar engine; the dense linear part is one
more fused vector op against a replicated weight vector.
"""

import os
import sys

import numpy as np


def _import_concourse():
    try:
        import concourse  # noqa: F401
    except ImportError:
        for p in ("/opt/trn_rl_repo", "/root/.axon_site/_ro/trn_rl_repo"):
            if os.path.isdir(p) and p not in sys.path:
                sys.path.insert(0, p)
    import concourse.bass as bass  # noqa: F401
    import concourse.mybir as mybir  # noqa: F401
    import concourse.tile as tile  # noqa: F401
    from concourse import bass_utils  # noqa: F401

    return bass, mybir, tile, bass_utils


# Problem constants (hardcoded per contract)
F = 20          # sparse fields
V = 10000       # vocab per field
VTOT = F * V    # 200000
D = 16          # embed dim
B = 16384       # batch
DD = 13         # dense features
N_CORES = 8
P = 128         # SBUF partitions

BPC = B // N_CORES          # 2048 samples per core
N_TILES = BPC // P          # 16 tiles of 128 samples
ROW = 384                   # gather row (bf16): 19*16 emb + wsp + pad (768 B)
EMB = (F - 1) * D           # 304
WSP = EMB                   # w_sparse slot index
NPAIR_ELEMS = (F * (F - 1) // 2) * D  # 3040 pair-product elements per sample
N_G = 4                     # batch-tiles per gather group
COLS_PER_WIN = 3            # fields per gather window (3*V < int16 max)
SINGLE_PACKET = False
N_QUEUES = 4

WINDOWS = [
    (c0, min(COLS_PER_WIN, F - c0)) for c0 in range(0, F, COLS_PER_WIN)
]
NW = len(WINDOWS)
MAXW = COLS_PER_WIN * 8 * N_G  # idx columns per (group, window) incl. 8x wrap


def _patch_queue_lanes():
    """Make Tile assign DMASW sem lanes per SWDGE queue (lane 2q/2q+1 for
    queue q) — the runtime locks each lane to one queue, but stock Tile
    round-robins lanes obliviously."""
    import concourse.tile_sem_assignment as tsa

    if getattr(tsa, "_ffm_queue_patch", False):
        return
    import concourse.mybir as mybir

    orig = tsa.TileClockTick._assign_tick

    def patched(self, inst):
        q = getattr(inst, "queue_num", None)
        if (
            q is not None
            and isinstance(inst, tsa.DMAInst)
            and inst.engine == mybir.EngineType.Pool
        ):
            state = getattr(self, "_ffm_perq", None)
            if state is None:
                state = {}
                self._ffm_perq = state
            self.next_sw_dma_idx = 2 * q + state.get(q, 0)
            orig(self, inst)
            state[q] = state.get(q, 0) ^ 1
            return
        orig(self, inst)

    tsa.TileClockTick._assign_tick = patched
    tsa._ffm_queue_patch = True


def _build_program(n_tiles=N_TILES, vtot=VTOT, n_g=N_G, for_sim=False):
    bass, mybir, tile, _ = _import_concourse()
    _patch_queue_lanes()

    v = vtot // F
    n_groups = n_tiles // n_g
    assert n_tiles % n_g == 0

    import concourse.bacc as bacc

    # Bacc (not plain Bass): its compile() runs generate_event_semaphores,
    # which splits multi-semaphore waits into InstEventSemaphore prefixes —
    # TRN2 instructions can carry only one inline wait — and inserts the
    # GPSIMD ucode library loads that dma_gather needs.
    nc = bacc.Bacc(None, target_bir_lowering=False, debug=for_sim,
                   num_swdge_queues=N_QUEUES)

    f32 = mybir.dt.float32
    bf16 = mybir.dt.bfloat16
    i16 = mybir.dt.int16
    mult = mybir.AluOpType.mult
    copy_f = mybir.ActivationFunctionType.Copy
    sigm_f = mybir.ActivationFunctionType.Sigmoid

    t2 = nc.dram_tensor("t2", [vtot, ROW], bf16, kind="ExternalInput")
    idxs = nc.dram_tensor("idxs", [P, n_groups, NW, MAXW], i16, kind="ExternalInput")
    dense_d = nc.dram_tensor("dense", [DD + 1, n_tiles, P], f32, kind="ExternalInput")
    wvec_d = nc.dram_tensor("wvec", [DD + 1, 1], f32, kind="ExternalInput")
    out = nc.dram_tensor("out", [P, n_tiles], f32, kind="ExternalOutput")

    with tile.TileContext(nc) as tc:
        with (
            tc.tile_pool(name="const", bufs=1) as cpool,
            tc.tile_pool(name="gather", bufs=2) as gpool,
            tc.tile_pool(name="scratch", bufs=2) as spool,
            tc.tile_pool(name="accp", bufs=4) as apool,
            tc.tile_pool(name="psum", bufs=2, space="PSUM") as pspool,
        ):
            idx_sb = cpool.tile([P, n_groups, NW, MAXW], i16)
            dense_sb = cpool.tile([DD + 1, n_tiles, P], f32)
            wvec_sb = cpool.tile([DD + 1, 1], f32)
            out_all = cpool.tile([P, n_tiles], f32)
            nc.sync.dma_start(out=idx_sb[:], in_=idxs[:])
            nc.sync.dma_start(out=dense_sb[:], in_=dense_d[:])
            nc.sync.dma_start(out=wvec_sb[:], in_=wvec_d[:])

            gather_seq = 0
            for g_i in range(n_groups):
                gall = gpool.tile([P, F, n_g, ROW], bf16, tag="gall")
                # reversed window order: the descending-i compute consumes
                # windows last-to-first, so emitting w6 first lets compute
                # start after the first gather of the group lands
                for wi, (c0, ncw) in reversed(list(enumerate(WINDOWS))):
                    nidx = ncw * n_g * P
                    nc.gpsimd.dma_gather(
                        gall[:, c0 : c0 + ncw, :, :].rearrange(
                            "p c n r -> p (c n) r"
                        ),
                        t2[c0 * v : (c0 + ncw) * v, :],
                        idx_sb[:, g_i, wi, : ncw * 8 * n_g],
                        nidx,
                        nidx,
                        ROW,
                        single_packet=SINGLE_PACKET,
                        queue_num=gather_seq % N_QUEUES,
                    )
                    gather_seq += 1

                for n in range(n_g):
                    tt = g_i * n_g + n
                    prod = spool.tile([P, NPAIR_ELEMS], bf16, tag="prod")
                    pcp = spool.tile([P, NPAIR_ELEMS], bf16, tag="pcp")
                    acc2 = spool.tile([P, 3], f32, tag="acc2")
                    acc = apool.tile([P, 3], f32, tag="acc")
                    pre = apool.tile([P, 1], f32, tag="pre")

                    # cross: per i, the pair products (i, j>i) as one plain
                    # tensor_tensor multiply into a slice of prod (eligible
                    # for the DVE 16-bit 2x perf mode). Descending i so each
                    # op needs at most one gather window the engine hasn't
                    # already waited for.
                    off = 0
                    for i in reversed(range(F - 1)):
                        cnt = F - 1 - i
                        x = gall[:, i, n, i * D : EMB].rearrange(
                            "p (j d) -> p j d", d=D
                        )
                        y = gall[:, i + 1 : F, n, i * D : (i + 1) * D]
                        nc.vector.tensor_tensor(
                            out=prod[:, off : off + cnt * D].rearrange(
                                "p (j d) -> p j d", d=D
                            ),
                            in0=x,
                            in1=y,
                            op=mult,
                        )
                        off += cnt * D

                    # cross sum: one ACT accumulate over all pair products
                    nc.scalar.activation(
                        out=pcp[:],
                        in_=prod[:],
                        func=copy_f,
                        accum_out=acc[:, 0:1],
                    )

                    # linear sparse: sum the 20 w_sparse slots (on DVE so
                    # gall's only readers are DVE ops -> fast buffer recycle)
                    wcp = spool.tile([P, F], bf16, tag="wcp")
                    wsp_slice = gall[:, :, n, WSP : WSP + 1].rearrange(
                        "p c one -> p (c one)"
                    )
                    nc.vector.scalar_tensor_tensor(
                        out=wcp[:],
                        in0=wsp_slice,
                        scalar=1.0,
                        in1=wsp_slice,
                        op0=mult,
                        op1=mybir.AluOpType.bypass,
                        accum_out=acc[:, 1:2],
                    )

                    # linear dense + bias on the (idle) tensor engine
                    ps = pspool.tile([P, 1], f32)
                    nc.tensor.matmul(
                        out=ps[:],
                        lhsT=dense_sb[:, tt, :],
                        rhs=wvec_sb[:, :1],
                        start=True,
                        stop=True,
                    )
                    nc.scalar.copy(out=acc[:, 2:3], in_=ps[:])

                    # total + sigmoid
                    nc.scalar.activation(
                        out=acc2[:], in_=acc[:], func=copy_f, accum_out=pre[:]
                    )
                    nc.scalar.activation(
                        out=out_all[:, tt : tt + 1], in_=pre[:], func=sigm_f
                    )

            nc.sync.dma_start(out=out[:], in_=out_all[:])

    nc.compile()
    return nc


_PROGRAM_CACHE = {}


def _get_program():
    if "nc" not in _PROGRAM_CACHE:
        _PROGRAM_CACHE["nc"] = _build_program()
    return _PROGRAM_CACHE["nc"]


def make_idx_array(sparse_core, n_tiles=N_TILES, n_g=N_G, v=V):
    """sparse_core: [BPC, F] local ids (< V). Returns [P, n_groups, NW, MAXW] i16.

    dma_gather consumes idx element i from [partition i%16, col i//16],
    replicated 8x down the partitions. Within one (group, window) gather,
    i = (c_local * n_g + n) * 128 + p maps to out slot [p, c_local, n].
    """
    n_groups = n_tiles // n_g
    spc = sparse_core.reshape(P, n_tiles, F)  # [p, tt, c], sample s = p*n_tiles+tt
    arr = np.zeros((P, n_groups, NW, MAXW), dtype=np.int16)
    for g_i in range(n_groups):
        for wi, (c0, ncw) in enumerate(WINDOWS):
            # vals[c_local, n, p]
            vals = spc[:, g_i * n_g : (g_i + 1) * n_g, c0 : c0 + ncw].transpose(
                2, 1, 0
            ).astype(np.int64)
            vals = vals + (np.arange(ncw, dtype=np.int64) * v)[:, None, None]
            flat = vals.reshape(-1).astype(np.int16)  # i-ordered
            m = len(flat) // 16
            wrap = np.tile(flat.reshape(m, 16).T, (8, 1))  # [128, m]
            arr[:, g_i, wi, :m] = wrap
    return arr


def _prep_inputs(dense_input, sparse_input, tables, w_dense, w_sparse, bias):
    import ml_dtypes

    dense_input = np.asarray(dense_input, dtype=np.float32)
    sparse_input = np.asarray(sparse_input)
    tables = np.asarray(tables, dtype=np.float32)
    w_dense = np.asarray(w_dense, dtype=np.float32)
    w_sparse = np.asarray(w_sparse, dtype=np.float32)
    bias = np.asarray(bias, dtype=np.float32)

    # T2[g] = [tables[t, g, :] for t != g//V] ++ [w_sparse[g]] ++ pad, bf16
    t2 = np.zeros((VTOT, ROW), dtype=np.float32)
    for c in range(F):
        sl = slice(c * V, (c + 1) * V)
        sel = [t for t in range(F) if t != c]
        t2[sl, :EMB] = tables[sel, sl, :].transpose(1, 0, 2).reshape(V, EMB)
    t2[:, WSP] = w_sparse[:, 0]
    t2 = t2.astype(ml_dtypes.bfloat16)

    sparse_i = sparse_input.astype(np.int64).reshape(N_CORES, BPC, F)
    dense_aug = np.concatenate(
        [dense_input, np.ones((B, 1), dtype=np.float32)], axis=1
    ).reshape(N_CORES, P, N_TILES, DD + 1)
    waug = np.concatenate([w_dense[:, 0], bias]).astype(np.float32)
    wvec = waug.reshape(DD + 1, 1)

    in_maps = []
    for k in range(N_CORES):
        in_maps.append(
            {
                "t2": t2,
                "idxs": make_idx_array(sparse_i[k]),
                "dense": np.ascontiguousarray(dense_aug[k].transpose(2, 1, 0)),
                "wvec": wvec,
            }
        )
    return in_maps


def kernel(dense_input, sparse_input, tables, w_dense, w_sparse, bias, _trace=False):
    _, _, _, bass_utils = _import_concourse()

    nc = _get_program()
    in_maps = _prep_inputs(dense_input, sparse_input, tables, w_dense, w_sparse, bias)
    res = bass_utils.run_bass_kernel_spmd(
        nc, in_maps, core_ids=list(range(N_CORES)), trace=_trace
    )
    outs = [res.results[k]["out"].reshape(BPC) for k in range(N_CORES)]
    full = np.concatenate(outs).reshape(B, 1).astype(np.float32)
    if _trace:
        return full, res
    return full


# revision 22
# speedup vs baseline: 2.7908x; 1.0898x over previous
"""FFM layer (field-aware factorization machine) on 8 Trainium2 cores.

Strategy: data-parallel over batch (2048 samples/core). The embedding tables
are re-laid-out on the host into one row per global vocab id g (owned by
exactly one field c = g // V): the 19 *other* fields' embeddings for that id,
plus the w_sparse value, padded to 384 bf16 (768 B, the dma_gather 256 B
granularity). Each (sample, field) lookup is then one contiguous gather row.

The gather uses nc.gpsimd.dma_gather (int16 indices). Indices must fit int16,
so gathers address vocab windows of 3 fields (3*10000 < 32767), with
window-relative indices. One gather instruction covers 3 fields x 4
batch-tiles = 1536 rows.

The FFM cross term for pair (i, j>i) is dot(row_i[block j], row_j[block i]);
per 128-sample tile it is computed as 19 fused multiply + per-partition-reduce
vector ops (scalar_tensor_tensor with accum_out). w_sparse sum and the final
reduction + sigmoid run on the scalar engine; the dense linear part is one
more fused vector op against a replicated weight vector.
"""

import os
import sys

import numpy as np


def _import_concourse():
    try:
        import concourse  # noqa: F401
    except ImportError:
        for p in ("/opt/trn_rl_repo", "/root/.axon_site/_ro/trn_rl_repo"):
            if os.path.isdir(p) and p not in sys.path:
                sys.path.insert(0, p)
    import concourse.bass as bass  # noqa: F401
    import concourse.mybir as mybir  # noqa: F401
    import concourse.tile as tile  # noqa: F401
    from concourse import bass_utils  # noqa: F401

    return bass, mybir, tile, bass_utils


# Problem constants (hardcoded per contract)
F = 20          # sparse fields
V = 10000       # vocab per field
VTOT = F * V    # 200000
D = 16          # embed dim
B = 16384       # batch
DD = 13         # dense features
N_CORES = 8
P = 128         # SBUF partitions

BPC = B // N_CORES          # 2048 samples per core
N_TILES = BPC // P          # 16 tiles of 128 samples
ROW = 384                   # gather row (bf16): 19*16 emb + wsp + pad (768 B)
EMB = (F - 1) * D           # 304
WSP = EMB                   # w_sparse slot index
NPAIR_ELEMS = (F * (F - 1) // 2) * D  # 3040 pair-product elements per sample
N_G = 2                     # batch-tiles per gather group
COLS_PER_WIN = 3            # fields per gather window (3*V < int16 max)
SINGLE_PACKET = False
N_QUEUES = 4

WINDOWS = [
    (c0, min(COLS_PER_WIN, F - c0)) for c0 in range(0, F, COLS_PER_WIN)
]
NW = len(WINDOWS)
MAXW = COLS_PER_WIN * 8 * N_G  # idx columns per (group, window) incl. 8x wrap


def _patch_queue_lanes():
    """Make Tile assign DMASW sem lanes per SWDGE queue (lane 2q/2q+1 for
    queue q) — the runtime locks each lane to one queue, but stock Tile
    round-robins lanes obliviously."""
    import concourse.tile_sem_assignment as tsa

    if getattr(tsa, "_ffm_queue_patch", False):
        return
    import concourse.mybir as mybir

    orig = tsa.TileClockTick._assign_tick

    def patched(self, inst):
        q = getattr(inst, "queue_num", None)
        if (
            q is not None
            and isinstance(inst, tsa.DMAInst)
            and inst.engine == mybir.EngineType.Pool
        ):
            state = getattr(self, "_ffm_perq", None)
            if state is None:
                state = {}
                self._ffm_perq = state
            self.next_sw_dma_idx = 2 * q + state.get(q, 0)
            orig(self, inst)
            state[q] = state.get(q, 0) ^ 1
            return
        orig(self, inst)

    tsa.TileClockTick._assign_tick = patched
    tsa._ffm_queue_patch = True


def _build_program(n_tiles=N_TILES, vtot=VTOT, n_g=N_G, for_sim=False):
    bass, mybir, tile, _ = _import_concourse()
    _patch_queue_lanes()

    v = vtot // F
    n_groups = n_tiles // n_g
    assert n_tiles % n_g == 0

    import concourse.bacc as bacc

    # Bacc (not plain Bass): its compile() runs generate_event_semaphores,
    # which splits multi-semaphore waits into InstEventSemaphore prefixes —
    # TRN2 instructions can carry only one inline wait — and inserts the
    # GPSIMD ucode library loads that dma_gather needs.
    nc = bacc.Bacc(None, target_bir_lowering=False, debug=for_sim,
                   num_swdge_queues=N_QUEUES)

    f32 = mybir.dt.float32
    bf16 = mybir.dt.bfloat16
    i16 = mybir.dt.int16
    mult = mybir.AluOpType.mult
    copy_f = mybir.ActivationFunctionType.Copy
    sigm_f = mybir.ActivationFunctionType.Sigmoid

    t2 = nc.dram_tensor("t2", [vtot, ROW], bf16, kind="ExternalInput")
    idxs = nc.dram_tensor("idxs", [P, n_groups, NW, MAXW], i16, kind="ExternalInput")
    dense_d = nc.dram_tensor("dense", [DD + 1, n_tiles, P], f32, kind="ExternalInput")
    wvec_d = nc.dram_tensor("wvec", [DD + 1, 1], f32, kind="ExternalInput")
    out = nc.dram_tensor("out", [P, n_tiles], f32, kind="ExternalOutput")

    with tile.TileContext(nc) as tc:
        with (
            tc.tile_pool(name="const", bufs=1) as cpool,
            tc.tile_pool(name="gather", bufs=2) as gpool,
            tc.tile_pool(name="scratch", bufs=2) as spool,
            tc.tile_pool(name="accp", bufs=4) as apool,
            tc.tile_pool(name="psum", bufs=2, space="PSUM") as pspool,
        ):
            idx_sb = cpool.tile([P, n_groups, NW, MAXW], i16)
            dense_sb = cpool.tile([DD + 1, n_tiles, P], f32)
            wvec_sb = cpool.tile([DD + 1, 1], f32)
            out_all = cpool.tile([P, n_tiles], f32)
            nc.sync.dma_start(out=idx_sb[:], in_=idxs[:])
            nc.sync.dma_start(out=dense_sb[:], in_=dense_d[:])
            nc.sync.dma_start(out=wvec_sb[:], in_=wvec_d[:])

            gather_seq = 0
            for g_i in range(n_groups):
                gall = gpool.tile([P, F, n_g, ROW], bf16, tag="gall")
                # reversed window order: the descending-i compute consumes
                # windows last-to-first, so emitting w6 first lets compute
                # start after the first gather of the group lands
                for wi, (c0, ncw) in reversed(list(enumerate(WINDOWS))):
                    nidx = ncw * n_g * P
                    nc.gpsimd.dma_gather(
                        gall[:, c0 : c0 + ncw, :, :].rearrange(
                            "p c n r -> p (c n) r"
                        ),
                        t2[c0 * v : (c0 + ncw) * v, :],
                        idx_sb[:, g_i, wi, : ncw * 8 * n_g],
                        nidx,
                        nidx,
                        ROW,
                        single_packet=SINGLE_PACKET,
                        queue_num=gather_seq % N_QUEUES,
                    )
                    gather_seq += 1

                for n in range(n_g):
                    tt = g_i * n_g + n
                    prod = spool.tile([P, NPAIR_ELEMS], bf16, tag="prod")
                    pcp = spool.tile([P, NPAIR_ELEMS], bf16, tag="pcp")
                    acc2 = spool.tile([P, 3], f32, tag="acc2")
                    acc = apool.tile([P, 3], f32, tag="acc")
                    pre = apool.tile([P, 1], f32, tag="pre")

                    # cross: per i, the pair products (i, j>i) as one plain
                    # tensor_tensor multiply into a slice of prod (eligible
                    # for the DVE 16-bit 2x perf mode). Descending i so each
                    # op needs at most one gather window the engine hasn't
                    # already waited for.
                    off = 0
                    for i in reversed(range(F - 1)):
                        cnt = F - 1 - i
                        x = gall[:, i, n, i * D : EMB].rearrange(
                            "p (j d) -> p j d", d=D
                        )
                        y = gall[:, i + 1 : F, n, i * D : (i + 1) * D]
                        nc.vector.tensor_tensor(
                            out=prod[:, off : off + cnt * D].rearrange(
                                "p (j d) -> p j d", d=D
                            ),
                            in0=x,
                            in1=y,
                            op=mult,
                        )
                        off += cnt * D

                    # cross sum: one ACT accumulate over all pair products
                    nc.scalar.activation(
                        out=pcp[:],
                        in_=prod[:],
                        func=copy_f,
                        accum_out=acc[:, 0:1],
                    )

                    # linear sparse: sum the 20 w_sparse slots (on DVE so
                    # gall's only readers are DVE ops -> fast buffer recycle)
                    wcp = spool.tile([P, F], bf16, tag="wcp")
                    wsp_slice = gall[:, :, n, WSP : WSP + 1].rearrange(
                        "p c one -> p (c one)"
                    )
                    nc.vector.scalar_tensor_tensor(
                        out=wcp[:],
                        in0=wsp_slice,
                        scalar=1.0,
                        in1=wsp_slice,
                        op0=mult,
                        op1=mybir.AluOpType.bypass,
                        accum_out=acc[:, 1:2],
                    )

                    # linear dense + bias on the (idle) tensor engine
                    ps = pspool.tile([P, 1], f32)
                    nc.tensor.matmul(
                        out=ps[:],
                        lhsT=dense_sb[:, tt, :],
                        rhs=wvec_sb[:, :1],
                        start=True,
                        stop=True,
                    )
                    nc.scalar.copy(out=acc[:, 2:3], in_=ps[:])

                    # total + sigmoid
                    nc.scalar.activation(
                        out=acc2[:], in_=acc[:], func=copy_f, accum_out=pre[:]
                    )
                    nc.scalar.activation(
                        out=out_all[:, tt : tt + 1], in_=pre[:], func=sigm_f
                    )

            nc.sync.dma_start(out=out[:], in_=out_all[:])

    nc.compile()
    return nc


_PROGRAM_CACHE = {}


def _get_program():
    if "nc" not in _PROGRAM_CACHE:
        _PROGRAM_CACHE["nc"] = _build_program()
    return _PROGRAM_CACHE["nc"]


def make_idx_array(sparse_core, n_tiles=N_TILES, n_g=N_G, v=V):
    """sparse_core: [BPC, F] local ids (< V). Returns [P, n_groups, NW, MAXW] i16.

    dma_gather consumes idx element i from [partition i%16, col i//16],
    replicated 8x down the partitions. Within one (group, window) gather,
    i = (c_local * n_g + n) * 128 + p maps to out slot [p, c_local, n].
    """
    n_groups = n_tiles // n_g
    spc = sparse_core.reshape(P, n_tiles, F)  # [p, tt, c], sample s = p*n_tiles+tt
    arr = np.zeros((P, n_groups, NW, MAXW), dtype=np.int16)
    for g_i in range(n_groups):
        for wi, (c0, ncw) in enumerate(WINDOWS):
            # vals[c_local, n, p]
            vals = spc[:, g_i * n_g : (g_i + 1) * n_g, c0 : c0 + ncw].transpose(
                2, 1, 0
            ).astype(np.int64)
            vals = vals + (np.arange(ncw, dtype=np.int64) * v)[:, None, None]
            flat = vals.reshape(-1).astype(np.int16)  # i-ordered
            m = len(flat) // 16
            wrap = np.tile(flat.reshape(m, 16).T, (8, 1))  # [128, m]
            arr[:, g_i, wi, :m] = wrap
    return arr


def _prep_inputs(dense_input, sparse_input, tables, w_dense, w_sparse, bias):
    import ml_dtypes

    dense_input = np.asarray(dense_input, dtype=np.float32)
    sparse_input = np.asarray(sparse_input)
    tables = np.asarray(tables, dtype=np.float32)
    w_dense = np.asarray(w_dense, dtype=np.float32)
    w_sparse = np.asarray(w_sparse, dtype=np.float32)
    bias = np.asarray(bias, dtype=np.float32)

    # T2[g] = [tables[t, g, :] for t != g//V] ++ [w_sparse[g]] ++ pad, bf16
    t2 = np.zeros((VTOT, ROW), dtype=np.float32)
    for c in range(F):
        sl = slice(c * V, (c + 1) * V)
        sel = [t for t in range(F) if t != c]
        t2[sl, :EMB] = tables[sel, sl, :].transpose(1, 0, 2).reshape(V, EMB)
    t2[:, WSP] = w_sparse[:, 0]
    t2 = t2.astype(ml_dtypes.bfloat16)

    sparse_i = sparse_input.astype(np.int64).reshape(N_CORES, BPC, F)
    dense_aug = np.concatenate(
        [dense_input, np.ones((B, 1), dtype=np.float32)], axis=1
    ).reshape(N_CORES, P, N_TILES, DD + 1)
    waug = np.concatenate([w_dense[:, 0], bias]).astype(np.float32)
    wvec = waug.reshape(DD + 1, 1)

    in_maps = []
    for k in range(N_CORES):
        in_maps.append(
            {
                "t2": t2,
                "idxs": make_idx_array(sparse_i[k]),
                "dense": np.ascontiguousarray(dense_aug[k].transpose(2, 1, 0)),
                "wvec": wvec,
            }
        )
    return in_maps


def kernel(dense_input, sparse_input, tables, w_dense, w_sparse, bias, _trace=False):
    _, _, _, bass_utils = _import_concourse()

    nc = _get_program()
    in_maps = _prep_inputs(dense_input, sparse_input, tables, w_dense, w_sparse, bias)
    res = bass_utils.run_bass_kernel_spmd(
        nc, in_maps, core_ids=list(range(N_CORES)), trace=_trace
    )
    outs = [res.results[k]["out"].reshape(BPC) for k in range(N_CORES)]
    full = np.concatenate(outs).reshape(B, 1).astype(np.float32)
    if _trace:
        return full, res
    return full
